# revision 29
# baseline (speedup 1.0000x reference)
"""Trainium2 Bass kernel for nn_K_attention_ex (gaussian-kernel residual attention).

Reference computation (per batch sample b):
    sq_i   = ||x_i||^2
    G      = x @ x^T                      (T,T) gram
    sqdist = relu(sq_i + sq_j - 2 G)
    K      = exp(-sqdist * r + m) * (1 - eye)
    out    = x + K @ x

Algebraic restructuring (exact up to fp rounding):
    K_full = beta * e_i * e_j * exp(2 r g_ij),   e = exp(-r*sq), beta = exp(m)
    out = (1-beta)*x + beta * e ⊙_row ( E @ (e ⊙_row x) ),  E = exp(2 r G)

Key structure (vs the 101us v1 kernel):
  * E = exp(2rG) is symmetric: only the upper-triangular block row-slabs
    are computed (gram in bf16) and exponentiated on ACT — halves the ACT
    exp work, which was the v1 bottleneck (75% busy).
  * The strictly-lower blocks are reconstructed with DMA-xbar transposes
    (dma_start_transpose, ~14ns per 16x128 tile) into a packed ET tile:
    no PE/ACT/DVE cycles spent on the mirror.
  * Y^T accumulates per 128-column block: 16 bf16 matmuls per block
    (rows a<=j from packed upper E, rows a>j from ET). Y emission lags
    the exp/xbar producer by YSHIFT steps to hide the ~2us DMA
    dispatch+transfer latency of the xbar mirror.
  * Y^T -> natural layout via DMA-xbar transposes (bf16), one per
    512-column group, pipelined with compute; PSUM evacuation of Y^T on
    the otherwise-idle GPSIMD engine; ||x||^2 partially on GPSIMD.
  * Both samples run in ONE merged software pipeline over 32 global
    steps (sample = step//16), so there is no drain/fill bubble at the
    sample boundary; the next sample's front-end (cast/transpose/prep)
    is emitted into the tail steps of the previous sample's loop.

bf16 is used for gram + Y matmul operands (output rel err ~4e-3, gate
2e-2); fp8 gram was tried and rejected: per-row quantization error of x
is amplified by the near-constant positive E into ~3e-2 output error.

Sharding: data-parallel over batch B=16 across 8 NeuronCores (2 samples each).
"""

import numpy as np

import concourse.bass as bass
import concourse.tile as tile
from concourse import bacc, mybir
from concourse.bass_utils import run_bass_kernel_spmd
from concourse.masks import make_identity

F32 = mybir.dt.float32
BF16 = mybir.dt.bfloat16
AF = mybir.ActivationFunctionType
MUL = mybir.AluOpType.mult
ADD = mybir.AluOpType.add

B, T, C = 16, 2048, 64
N_CORES = 8
BPC = B // N_CORES          # samples per core
NK = T // 128               # 16 row-blocks of 128
YSHIFT = 2                  # Y column lag behind exp/xbar (hides xbar latency)
OSHIFT = YSHIFT + 2         # output-group lag behind Y columns
SOFF = NK - 3               # step offset between sample pipelines (overlap=3)

# Packed upper-triangular E storage: row j holds blocks (j, j..15),
# width (16-j)*128, at free-offset EOF[j].
EOF = []
_o = 0
for _j in range(NK):
    EOF.append(_o)
    _o += (NK - _j) * 128
E_W = _o                     # 17408 elems/partition (bf16 -> 34 KiB)

# ET packing: row j's off-diag blocks (j,k), k>j, transposed, at slot
# ET_OFF[j] + (k-j-1).
ET_OFF = []
_o = 0
for _j in range(NK):
    ET_OFF.append(_o)
    _o += (NK - 1) - _j
N_ET = _o                    # 120


def build_nc(reps=1, stages='all'):
    nc = bacc.Bacc("TRN2", target_bir_lowering=False, debug=False, num_devices=N_CORES)
    x_in = nc.dram_tensor("x", [BPC, T, C], F32, kind="ExternalInput")
    r_in = nc.dram_tensor("r_sigma", [1], F32, kind="ExternalInput")
    m_in = nc.dram_tensor("margin", [1], F32, kind="ExternalInput")
    o_out = nc.dram_tensor("out", [BPC, T, C], F32, kind="ExternalOutput")

    with tile.TileContext(nc) as tc:
        if reps == 1:
            _body(tc, o_out.ap(), x_in.ap(), r_in.ap(), m_in.ap(), stages)
        else:
            with tc.For_i(0, reps, 1):
                _body(tc, o_out.ap(), x_in.ap(), r_in.ap(), m_in.ap(), stages)
    nc.compile()
    return nc


LEVELS = {'xload': 0, 'xt': 1, 'prep': 2, 'gram': 3, 'exp': 4, 'y': 5, 'all': 6}


def _body(tc, out_ap, x_ap, r_ap, m_ap, stages='all'):
    lvl = LEVELS[stages]
    do = lambda name: lvl >= LEVELS.get(name, 6)
    nc = tc.nc
    with (
        tc.tile_pool(name="consts", bufs=1) as consts,
        tc.tile_pool(name="sx", bufs=2) as sx,
        tc.tile_pool(name="ebig", bufs=2) as ebig,
        tc.tile_pool(name="psG", bufs=3, space="PSUM") as psG,
        tc.tile_pool(name="psY", bufs=2, space="PSUM") as psY,
    ):
        # ---- one-time constants ----
        identb = consts.tile([128, 128], BF16)
        make_identity(nc, identb)
        rb = consts.tile([128, 1], F32)
        nc.gpsimd.dma_start(out=rb, in_=r_ap.to_broadcast((128, 1)))
        mb = consts.tile([128, 1], F32)
        nc.gpsimd.dma_start(out=mb, in_=m_ap.to_broadcast((128, 1)))
        negr = consts.tile([128, 1], F32)
        nc.vector.tensor_scalar_mul(out=negr, in0=rb, scalar1=-1.0)
        s2r = consts.tile([128, 1], F32)
        nc.vector.tensor_scalar_mul(out=s2r, in0=rb, scalar1=2.0)
        beta = consts.tile([128, 1], F32)
        nc.scalar.activation(out=beta, in_=mb, func=AF.Exp)
        alpha = consts.tile([128, 1], F32)  # 1 - beta
        nc.vector.tensor_scalar(
            out=alpha, in0=beta, scalar1=-1.0, scalar2=1.0, op0=MUL, op1=ADD,
        )

        # prefetch all samples' inputs up front
        x_sbs = []
        for s in range(BPC):
            xv = x_ap[s].rearrange("(p k) c -> p k c", p=128)
            x_sb = sx.tile([128, NK, C], F32, tag="x_sb", name=f"x_sb_{s}")
            nc.sync.dma_start(out=x_sb[:, 0:8, :], in_=xv[:, 0:8, :])
            nc.gpsimd.dma_start(out=x_sb[:, 8:NK, :], in_=xv[:, 8:NK, :])
            x_sbs.append(x_sb)

        # ---- per-sample state ----
        st = [
            {
                "x_bf": None, "xTb": None, "xsq": None, "sq": None,
                "f": None, "xs": None, "ax": None,
                "E": None, "ET": None, "YTsb": None, "ynat": None,
                "outsb": None, "yts": [None] * 4, "gs": None,
            }
            for _ in range(BPC)
        ]
        dma_legs = [nc.sync, nc.sync, nc.sync, nc.scalar]

        # ---- front-end emitters ----
        def emit_sq_mul(s):
            if not do('prep'):
                return
            xsq = sx.tile([128, NK, C], F32, tag="xsq", name=f"xsq_{s}")
            nc.vector.tensor_mul(xsq, x_sbs[s], x_sbs[s])
            st[s]["xsq"] = xsq

        def emit_sq_reduce(s):
            if not do('prep'):
                return
            sq = sx.tile([128, NK], F32, tag="sq", name=f"sq_{s}")
            nc.vector.reduce_sum(out=sq, in_=st[s]["xsq"], axis=mybir.AxisListType.X)
            st[s]["sq"] = sq

        def emit_cast(s):
            x_bf = sx.tile([128, NK, C], BF16, tag="x_bf", name=f"x_bf_{s}")
            # two halves, following the two input-DMA legs (subtile deps)
            nc.vector.tensor_copy(out=x_bf[:, 0:8, :], in_=x_sbs[s][:, 0:8, :])
            nc.vector.tensor_copy(out=x_bf[:, 8:NK, :], in_=x_sbs[s][:, 8:NK, :])
            st[s]["x_bf"] = x_bf
            st[s]["xTb"] = sx.tile([64, T], BF16, tag="xTb", name=f"xTb_{s}")

        def emit_transp(s, g):
            if not do('xt'):
                return
            x_bf, xTb = st[s]["x_bf"], st[s]["xTb"]
            xtr = psG.tile([64, 4, 128], BF16, tag="G", name=f"xtr_{s}_{g}")
            for kk in range(4):
                k = 4 * g + kk
                nc.tensor.transpose(
                    out=xtr[:, kk, :], in_=x_bf[:, k, :], identity=identb
                )
            nc.vector.tensor_copy(
                out=xTb[:, 512 * g : 512 * (g + 1)],
                in_=xtr.rearrange("p a b -> p (a b)"),
            )

        def emit_xsf(s):
            if not do('prep'):
                return
            e = sx.tile([128, NK], F32, tag="e", name=f"e_{s}")
            nc.scalar.activation(out=e, in_=st[s]["sq"], func=AF.Exp, scale=negr)
            f = sx.tile([128, NK], F32, tag="f", name=f"f_{s}")
            nc.vector.tensor_scalar_mul(out=f, in0=e, scalar1=beta)
            xs_bf = sx.tile([128, NK, C], BF16, tag="xs_bf", name=f"xs_bf_{s}")
            for k in range(NK):
                nc.vector.tensor_scalar_mul(
                    out=xs_bf[:, k, :], in0=st[s]["x_bf"][:, k, :],
                    scalar1=e[:, k : k + 1],
                )
            st[s]["f"], st[s]["xs"] = f, xs_bf

        def emit_ax(s):
            if not do('prep'):
                return
            ax = sx.tile([128, NK, C], F32, tag="ax", name=f"ax_{s}")
            nc.vector.tensor_scalar_mul(out=ax, in0=x_sbs[s], scalar1=alpha)
            st[s]["ax"] = ax

        def emit_front(s):
            emit_cast(s)
            emit_transp(s, 0)
            emit_transp(s, 1)
            emit_sq_mul(s)
            emit_sq_reduce(s)
            emit_transp(s, 2)
            emit_transp(s, 3)
            emit_xsf(s)

        def alloc_main(s):
            st[s]["E"] = ebig.tile([128, E_W], BF16, tag="E", name=f"E_{s}")
            st[s]["ET"] = ebig.tile([128, N_ET, 128], BF16, tag="ET", name=f"ET_{s}")
            st[s]["YTsb"] = sx.tile([64, T], BF16, tag="YTsb", name=f"YTsb_{s}")
            st[s]["ynat"] = sx.tile([128, NK, C], BF16, tag="ynat", name=f"ynat_{s}")
            st[s]["outsb"] = sx.tile([128, NK, C], F32, tag="outsb", name=f"outsb_{s}")

        # ---- main-pipeline emitters ----
        def emit_gram(s, j):
            """Upper row-slab j of G = x x^T (bf16), 1024-col G tiles."""
            xTb = st[s]["xTb"]
            lhsT = xTb[:, 128 * j : 128 * (j + 1)]
            gs = []
            c0 = 128 * j
            while c0 < T:
                w = min(1024, T - c0)
                G = psG.tile([128, 1024], F32, tag="G", name=f"G_{s}_{j}_{c0}")
                for q0 in range(0, w, 512):
                    qw = min(512, w - q0)
                    nc.tensor.matmul(
                        out=G[:, q0 : q0 + qw],
                        lhsT=lhsT,
                        rhs=xTb[:, c0 + q0 : c0 + q0 + qw],
                        start=True,
                        stop=True,
                    )
                gs.append((G, c0, w))
                c0 += w
            st[s]["gs"] = gs

        def emit_exp(s, j):
            E = st[s]["E"]
            for (G, c0, w) in st[s]["gs"]:
                o0 = EOF[j] + (c0 - 128 * j)
                if do('exp'):
                    nc.scalar.activation(
                        out=E[:, o0 : o0 + w], in_=G[:, 0:w], func=AF.Exp, scale=s2r,
                    )
                else:
                    nc.scalar.activation(
                        out=E[:, o0 : o0 + w], in_=G[:, 0:w], func=AF.Copy,
                    )

        def emit_xbar(s, j):
            if j >= NK - 1:
                return
            E, ET = st[s]["E"], st[s]["ET"]
            nb = (NK - 1) - j
            nc.sync.dma_start_transpose(
                out=ET[:, ET_OFF[j] : ET_OFF[j] + nb, :],
                in_=E[:, EOF[j] + 128 : EOF[j] + (NK - j) * 128],
            )

        def emit_y(s, j):
            # Y^T column-block j into psum group tile g = j//4.
            E, ET, xs_bf, yts = st[s]["E"], st[s]["ET"], st[s]["xs"], st[s]["yts"]
            g = j // 4
            if yts[g] is None:
                yts[g] = psY.tile([64, 512], F32, tag="YT", name=f"YT_{s}_{g}")
            yt = yts[g]
            q = 128 * (j % 4)
            for i, a in enumerate(range(NK)):
                rhs = (
                    E[:, EOF[a] + 128 * (j - a) : EOF[a] + 128 * (j - a) + 128]
                    if a <= j
                    else ET[:, ET_OFF[j] + (a - j - 1), :]
                )
                nc.tensor.matmul(
                    out=yt[:, q : q + 128],
                    lhsT=xs_bf[:, a, :],
                    rhs=rhs,
                    start=(i == 0),
                    stop=(i == NK - 1),
                )
            # NOTE: PSUM evacuation must be on DVE/ACT — GPSIMD cannot
            # access PSUM on real hardware (BIR verifier rejects it).
            last_grp = s == BPC - 1 and g == 3
            if last_grp:
                # drain: evacuate per column so the final output chain
                # (evac -> xbar -> combine -> store) starts ASAP
                nc.vector.tensor_copy(
                    out=st[s]["YTsb"][:, 128 * j : 128 * (j + 1)],
                    in_=yt[:, q : q + 128],
                )
                if j % 4 == 3:
                    yts[g] = None
            elif j % 4 == 3:
                nc.vector.tensor_copy(
                    out=st[s]["YTsb"][:, 512 * g : 512 * (g + 1)], in_=yt
                )
                yts[g] = None

        def emit_out(s, g, kcols=None):
            # group g: Y^T slab -> natural layout, combine, store.
            # kcols: emit a single column k (drain mode) instead of the group.
            if not do('all'):
                return
            YTsb, ynat, outsb = st[s]["YTsb"], st[s]["ynat"], st[s]["outsb"]
            f, ax = st[s]["f"], st[s]["ax"]
            ks = kcols if kcols is not None else list(range(4 * g, 4 * (g + 1)))
            c0, c1 = 128 * ks[0], 128 * (ks[-1] + 1)
            nc.sync.dma_start_transpose(
                out=ynat[:, ks[0] : ks[-1] + 1, :],
                in_=YTsb[:, c0:c1],
            )
            for k in ks:
                nc.vector.scalar_tensor_tensor(
                    out=outsb[:, k, :], in0=ynat[:, k, :], scalar=f[:, k : k + 1],
                    in1=ax[:, k, :], op0=MUL, op1=ADD,
                )
            ov = out_ap[s].rearrange("(p k) c -> p k c", p=128)
            dma_legs[g].dma_start(
                out=ov[:, ks[0] : ks[-1] + 1, :], in_=outsb[:, ks[0] : ks[-1] + 1, :]
            )

        # ---- merged pipeline over all samples ----
        emit_front(0)

        if not do('gram'):
            for s in range(1, BPC):
                emit_front(s)
            if do('all'):
                for s in range(BPC):
                    emit_ax(s)
                    st[s]["YTsb"] = sx.tile([64, T], BF16, tag="YTsb", name=f"YTsb_{s}")
                    st[s]["ynat"] = sx.tile([128, NK, C], BF16, tag="ynat", name=f"ynat_{s}")
                    st[s]["outsb"] = sx.tile([128, NK, C], F32, tag="outsb", name=f"outsb_{s}")
                    nc.vector.memset(st[s]["YTsb"], 0.0)
                    for g in range(4):
                        emit_out(s, g)
            return

        # Sample s's pipeline runs at local step j = t - s*SOFF, so
        # consecutive samples overlap by NK-SOFF steps: the next sample's
        # wide early gram rows interleave with this sample's Y tail.
        T_END = (BPC - 1) * SOFF + NK + OSHIFT
        for t in range(-1, T_END + 1):
            for s in range(BPC):
                j = t - s * SOFF
                if j < -1 or j > NK + OSHIFT:
                    continue
                if j == -1:
                    alloc_main(s)
                    emit_gram(s, 0)
                    continue
                last = s == BPC - 1
                if j < NK:
                    emit_exp(s, j)
                jy = j - YSHIFT
                if do('y') and 0 <= jy < NK:
                    emit_y(s, jy)
                jg = j + 1
                if jg < NK:
                    emit_gram(s, jg)
                if j < NK:
                    emit_xbar(s, j)
                # hooks: per-sample ax + next sample's front-end
                if j == 1:
                    emit_ax(s)
                sn = s + 1
                if sn < BPC:
                    if j == SOFF - 9:
                        emit_sq_mul(sn)
                        emit_cast(sn)
                    elif SOFF - 8 <= j <= SOFF - 5:
                        emit_transp(sn, j - (SOFF - 8))
                    elif j == SOFF - 4:
                        emit_sq_reduce(sn)
                    elif j == SOFF - 3:
                        emit_xsf(sn)
                if do('y'):
                    if last:
                        # drain: per-column output chain for the final group
                        jo2 = j - (YSHIFT + 1)
                        if 12 <= jo2 < NK:
                            emit_out(s, 3, [jo2])
                    jo = j - OSHIFT
                    if 0 <= jo < NK and jo % 4 == 3 and not (last and jo >= 12):
                        emit_out(s, jo // 4)

        if do('all') and not do('y'):
            for s in range(BPC):
                nc.vector.memset(st[s]["YTsb"], 0.0)
                for g in range(4):
                    emit_out(s, g)


_NC_CACHE = {}


def _get_nc(reps=1, stages='all'):
    key = (reps, stages)
    if key not in _NC_CACHE:
        _NC_CACHE[key] = build_nc(reps, stages)
    return _NC_CACHE[key]


def _run(x, r_sigma, margin, trace=False, reps=1, stages='all'):
    nc = _get_nc(reps, stages)
    x = np.ascontiguousarray(np.asarray(x, dtype=np.float32))
    r_sigma = np.ascontiguousarray(np.asarray(r_sigma, dtype=np.float32))
    margin = np.ascontiguousarray(np.asarray(margin, dtype=np.float32))
    in_maps = [
        {
            "x": np.ascontiguousarray(x[c * BPC : (c + 1) * BPC]),
            "r_sigma": r_sigma,
            "margin": margin,
        }
        for c in range(N_CORES)
    ]
    res = run_bass_kernel_spmd(nc, in_maps, core_ids=list(range(N_CORES)), trace=trace)
    out = np.concatenate([res.results[c]["out"] for c in range(N_CORES)], axis=0)
    return out, res


def kernel(x, r_sigma, margin):
    out, _ = _run(x, r_sigma, margin, trace=False)
    return out


# revision 38
# speedup vs baseline: 2.2458x; 2.2458x over previous
"""Trainium2 Bass kernel for nn_K_attention_ex (gaussian-kernel residual attention).

Reference computation (per batch sample b):
    sq_i   = ||x_i||^2
    G      = x @ x^T                      (T,T) gram
    sqdist = relu(sq_i + sq_j - 2 G)
    K      = exp(-sqdist * r + m) * (1 - eye)
    out    = x + K @ x

Algebraic restructuring (exact up to fp rounding):
    K_full = beta * e_i * e_j * exp(2 r g_ij),   e = exp(-r*sq), beta = exp(m)
    out = (1-beta)*x + beta * e ⊙_row ( E @ (e ⊙_row x) ),  E = exp(2 r G)

Key structure (vs the 101us v1 kernel):
  * E = exp(2rG) is symmetric: only the upper-triangular block row-slabs
    are computed (gram in bf16) and exponentiated on ACT — halves the ACT
    exp work, which was the v1 bottleneck (75% busy).
  * The strictly-lower blocks are reconstructed with DMA-xbar transposes
    (dma_start_transpose, ~14ns per 16x128 tile) into a packed ET tile:
    no PE/ACT/DVE cycles spent on the mirror.
  * Y^T accumulates per 128-column block: 16 bf16 matmuls per block
    (rows a<=j from packed upper E, rows a>j from ET). Y emission lags
    the exp/xbar producer by YSHIFT steps to hide the ~2us DMA
    dispatch+transfer latency of the xbar mirror.
  * Y^T -> natural layout via DMA-xbar transposes (bf16), one per
    512-column group, pipelined with compute; PSUM evacuation of Y^T on
    the otherwise-idle GPSIMD engine; ||x||^2 partially on GPSIMD.
  * Both samples run in ONE merged software pipeline over 32 global
    steps (sample = step//16), so there is no drain/fill bubble at the
    sample boundary; the next sample's front-end (cast/transpose/prep)
    is emitted into the tail steps of the previous sample's loop.

bf16 is used for gram + Y matmul operands (output rel err ~4e-3, gate
2e-2); fp8 gram was tried and rejected: per-row quantization error of x
is amplified by the near-constant positive E into ~3e-2 output error.

Sharding: data-parallel over batch B=16 across 8 NeuronCores (2 samples each).
"""

import numpy as np

import concourse.bass as bass
import concourse.tile as tile
from concourse import bacc, mybir
from concourse.bass_utils import run_bass_kernel_spmd
from concourse.masks import make_identity

F32 = mybir.dt.float32
BF16 = mybir.dt.bfloat16
AF = mybir.ActivationFunctionType
MUL = mybir.AluOpType.mult
ADD = mybir.AluOpType.add

B, T, C = 16, 2048, 64
N_CORES = 8
BPC = B // N_CORES          # samples per core
NK = T // 128               # 16 row-blocks of 128
YSHIFT = 2                  # Y column lag behind exp/xbar (hides xbar latency)
OSHIFT = YSHIFT + 2         # output-group lag behind Y columns
SOFF = NK - 3               # step offset between sample pipelines (overlap=3)
YMODE = "wide"              # "wide": stationary-major wide-N Y matmuls
GCHUNK = 512                # gram tile width (psum bank budget ties to YMODE)

# Packed upper-triangular E storage: row j holds blocks (j, j..15),
# width (16-j)*128, at free-offset EOF[j].
EOF = []
_o = 0
for _j in range(NK):
    EOF.append(_o)
    _o += (NK - _j) * 128
E_W = _o                     # 17408 elems/partition (bf16 -> 34 KiB)

# ET packing: row j's off-diag blocks (j,k), k>j, transposed, at slot
# ET_OFF[j] + (k-j-1).
ET_OFF = []
_o = 0
for _j in range(NK):
    ET_OFF.append(_o)
    _o += (NK - 1) - _j
N_ET = _o                    # 120


def build_nc(reps=1, stages='all'):
    nc = bacc.Bacc("TRN2", target_bir_lowering=False, debug=False, num_devices=N_CORES)
    x_in = nc.dram_tensor("x", [BPC, T, C], F32, kind="ExternalInput")
    r_in = nc.dram_tensor("r_sigma", [1], F32, kind="ExternalInput")
    m_in = nc.dram_tensor("margin", [1], F32, kind="ExternalInput")
    o_out = nc.dram_tensor("out", [BPC, T, C], F32, kind="ExternalOutput")

    with tile.TileContext(nc) as tc:
        if reps == 1:
            _body(tc, o_out.ap(), x_in.ap(), r_in.ap(), m_in.ap(), stages)
        else:
            with tc.For_i(0, reps, 1):
                _body(tc, o_out.ap(), x_in.ap(), r_in.ap(), m_in.ap(), stages)
    nc.compile()
    return nc


LEVELS = {'xload': 0, 'xt': 1, 'prep': 2, 'gram': 3, 'exp': 4, 'xbar': 5, 'y': 6, 'all': 7}


def _body(tc, out_ap, x_ap, r_ap, m_ap, stages='all'):
    lvl = LEVELS[stages]
    do = lambda name: lvl >= LEVELS.get(name, 6)
    nc = tc.nc
    with (
        tc.tile_pool(name="consts", bufs=1) as consts,
        tc.tile_pool(name="sx", bufs=2) as sx,
        tc.tile_pool(name="ebig", bufs=2) as ebig,
        tc.tile_pool(name="psG", bufs=4 if YMODE == "wide" else 3, space="PSUM") as psG,
        tc.tile_pool(name="psY", bufs=1 if YMODE == "wide" else 2, space="PSUM") as psY,
    ):
        # ---- one-time constants ----
        identb = consts.tile([128, 128], BF16)
        make_identity(nc, identb)
        rb = consts.tile([128, 1], F32)
        nc.gpsimd.dma_start(out=rb, in_=r_ap.to_broadcast((128, 1)))
        mb = consts.tile([128, 1], F32)
        nc.gpsimd.dma_start(out=mb, in_=m_ap.to_broadcast((128, 1)))
        negr = consts.tile([128, 1], F32)
        nc.vector.tensor_scalar_mul(out=negr, in0=rb, scalar1=-1.0)
        s2r = consts.tile([128, 1], F32)
        nc.vector.tensor_scalar_mul(out=s2r, in0=rb, scalar1=2.0)
        beta = consts.tile([128, 1], F32)
        nc.scalar.activation(out=beta, in_=mb, func=AF.Exp)
        alpha = consts.tile([128, 1], F32)  # 1 - beta
        nc.vector.tensor_scalar(
            out=alpha, in0=beta, scalar1=-1.0, scalar2=1.0, op0=MUL, op1=ADD,
        )

        # prefetch all samples' inputs up front
        x_sbs = []
        for s in range(BPC):
            xv = x_ap[s].rearrange("(p k) c -> p k c", p=128)
            x_sb = sx.tile([128, NK, C], F32, tag="x_sb", name=f"x_sb_{s}")
            nc.sync.dma_start(out=x_sb[:, 0:8, :], in_=xv[:, 0:8, :])
            nc.gpsimd.dma_start(out=x_sb[:, 8:NK, :], in_=xv[:, 8:NK, :])
            x_sbs.append(x_sb)

        # ---- per-sample state ----
        st = [
            {
                "x_bf": None, "xTb": None, "xsq": None, "sq": None,
                "f": None, "xs": None, "ax": None,
                "E": None, "ET": None, "YTsb": None, "ynat": None,
                "outsb": None, "yts": [None] * 4, "gs": None,
            }
            for _ in range(BPC)
        ]
        dma_legs = [nc.sync, nc.sync, nc.sync, nc.scalar]

        # ---- front-end emitters ----
        def emit_sq_mul(s):
            if not do('prep'):
                return
            xsq = sx.tile([128, NK, C], F32, tag="xsq", name=f"xsq_{s}")
            nc.vector.tensor_mul(xsq, x_sbs[s], x_sbs[s])
            st[s]["xsq"] = xsq

        def emit_sq_reduce(s):
            if not do('prep'):
                return
            sq = sx.tile([128, NK], F32, tag="sq", name=f"sq_{s}")
            nc.vector.reduce_sum(out=sq, in_=st[s]["xsq"], axis=mybir.AxisListType.X)
            st[s]["sq"] = sq

        def emit_cast(s):
            x_bf = sx.tile([128, NK, C], BF16, tag="x_bf", name=f"x_bf_{s}")
            # two halves, following the two input-DMA legs (subtile deps)
            nc.vector.tensor_copy(out=x_bf[:, 0:8, :], in_=x_sbs[s][:, 0:8, :])
            nc.vector.tensor_copy(out=x_bf[:, 8:NK, :], in_=x_sbs[s][:, 8:NK, :])
            st[s]["x_bf"] = x_bf
            st[s]["xTb"] = sx.tile([64, T], BF16, tag="xTb", name=f"xTb_{s}")

        def emit_transp(s, g):
            if not do('xt'):
                return
            x_bf, xTb = st[s]["x_bf"], st[s]["xTb"]
            xtr = psG.tile([64, 4, 128], BF16, tag="G", name=f"xtr_{s}_{g}")
            for kk in range(4):
                k = 4 * g + kk
                nc.tensor.transpose(
                    out=xtr[:, kk, :], in_=x_bf[:, k, :], identity=identb
                )
            nc.vector.tensor_copy(
                out=xTb[:, 512 * g : 512 * (g + 1)],
                in_=xtr.rearrange("p a b -> p (a b)"),
            )

        def emit_xsf(s):
            if not do('prep'):
                return
            e = sx.tile([128, NK], F32, tag="e", name=f"e_{s}")
            nc.scalar.activation(out=e, in_=st[s]["sq"], func=AF.Exp, scale=negr)
            f = sx.tile([128, NK], F32, tag="f", name=f"f_{s}")
            nc.vector.tensor_scalar_mul(out=f, in0=e, scalar1=beta)
            xs_bf = sx.tile([128, NK, C], BF16, tag="xs_bf", name=f"xs_bf_{s}")
            for k in range(NK):
                nc.vector.tensor_scalar_mul(
                    out=xs_bf[:, k, :], in0=st[s]["x_bf"][:, k, :],
                    scalar1=e[:, k : k + 1],
                )
            st[s]["f"], st[s]["xs"] = f, xs_bf

        def emit_ax(s):
            if not do('prep'):
                return
            ax = sx.tile([128, NK, C], F32, tag="ax", name=f"ax_{s}")
            nc.vector.tensor_scalar_mul(out=ax, in0=x_sbs[s], scalar1=alpha)
            st[s]["ax"] = ax

        def emit_front(s):
            emit_cast(s)
            emit_transp(s, 0)
            emit_transp(s, 1)
            emit_sq_mul(s)
            emit_sq_reduce(s)
            emit_transp(s, 2)
            emit_transp(s, 3)
            emit_xsf(s)

        def alloc_main(s):
            st[s]["E"] = ebig.tile([128, E_W], BF16, tag="E", name=f"E_{s}")
            st[s]["ET"] = ebig.tile([128, N_ET, 128], BF16, tag="ET", name=f"ET_{s}")
            st[s]["YTsb"] = sx.tile([64, T], BF16, tag="YTsb", name=f"YTsb_{s}")
            st[s]["ynat"] = sx.tile([128, NK, C], BF16, tag="ynat", name=f"ynat_{s}")
            st[s]["outsb"] = sx.tile([128, NK, C], F32, tag="outsb", name=f"outsb_{s}")

        # ---- main-pipeline emitters ----
        def emit_gram(s, j):
            """Upper row-slab j of G = x x^T (bf16), 1024-col G tiles."""
            xTb = st[s]["xTb"]
            lhsT = xTb[:, 128 * j : 128 * (j + 1)]
            gs = []
            c0 = 128 * j
            while c0 < T:
                w = min(GCHUNK, T - c0)
                G = psG.tile([128, GCHUNK], F32, tag="G", name=f"G_{s}_{j}_{c0}")
                for q0 in range(0, w, 512):
                    qw = min(512, w - q0)
                    nc.tensor.matmul(
                        out=G[:, q0 : q0 + qw],
                        lhsT=lhsT,
                        rhs=xTb[:, c0 + q0 : c0 + q0 + qw],
                        start=True,
                        stop=True,
                    )
                gs.append((G, c0, w))
                c0 += w
            st[s]["gs"] = gs

        def emit_exp(s, j):
            E = st[s]["E"]
            for (G, c0, w) in st[s]["gs"]:
                o0 = EOF[j] + (c0 - 128 * j)
                if do('exp'):
                    nc.scalar.activation(
                        out=E[:, o0 : o0 + w], in_=G[:, 0:w], func=AF.Exp, scale=s2r,
                    )
                else:
                    nc.scalar.activation(
                        out=E[:, o0 : o0 + w], in_=G[:, 0:w], func=AF.Copy,
                    )

        def emit_xbar(s, j):
            if j >= NK - 1 or not do('xbar'):
                return
            E, ET = st[s]["E"], st[s]["ET"]
            nb = (NK - 1) - j
            nc.sync.dma_start_transpose(
                out=ET[:, ET_OFF[j] : ET_OFF[j] + nb, :],
                in_=E[:, EOF[j] + 128 : EOF[j] + (NK - j) * 128],
            )

        def emit_ya(s, a):
            # Stationary-major Y: one stationary xs_a, wide-N matmuls.
            #   upper: columns a..15 from the packed E slab (chunks <=512)
            #   mirror: columns j<a from ET (N=128 each, same stationary)
            # All columns complete at a=15; group evacs interleave there.
            E, ET, xs_bf = st[s]["E"], st[s]["ET"], st[s]["xs"]
            if st[s]["yts"][0] is None:
                st[s]["yts"][0] = psY.tile([64, T], F32, tag="YT", name=f"YT_{s}")
            yt = st[s]["yts"][0]
            lhsT = xs_bf[:, a, :]
            last = a == NK - 1

            def upper_chunks():
                c0 = 128 * a
                while c0 < T:
                    w = min(512 - (c0 % 512), T - c0)
                    nc.tensor.matmul(
                        out=yt[:, c0 : c0 + w],
                        lhsT=lhsT,
                        rhs=E[:, EOF[a] + (c0 - 128 * a) : EOF[a] + (c0 - 128 * a) + w],
                        start=(a == 0),
                        stop=last,
                    )
                    c0 += w

            def mirror_block(j, stop):
                # stop only on a 512-col zero-region's final matmul: PSUM
                # group tracking is per 2KB bank, not per 128-col block
                nc.tensor.matmul(
                    out=yt[:, 128 * j : 128 * (j + 1)],
                    lhsT=lhsT,
                    rhs=ET[:, ET_OFF[j] + (a - j - 1), :],
                    start=False,
                    stop=stop,
                )

            if not last:
                upper_chunks()
                for j in range(a):
                    mirror_block(j, False)
            else:
                # column order so group evacuations can chase completion
                for j in range(NK - 1):
                    mirror_block(j, j % 4 == 3)
                    if j % 4 == 3:
                        g = j // 4
                        nc.vector.tensor_copy(
                            out=st[s]["YTsb"][:, 512 * g : 512 * (g + 1)],
                            in_=yt[:, 512 * g : 512 * (g + 1)],
                        )
                upper_chunks()
                nc.vector.tensor_copy(
                    out=st[s]["YTsb"][:, 1536:2048], in_=yt[:, 1536:2048]
                )
                st[s]["yts"][0] = None

        def emit_y(s, j):
            # Y^T column-block j into psum group tile g = j//4.
            E, ET, xs_bf, yts = st[s]["E"], st[s]["ET"], st[s]["xs"], st[s]["yts"]
            g = j // 4
            if yts[g] is None:
                yts[g] = psY.tile([64, 512], F32, tag="YT", name=f"YT_{s}_{g}")
            yt = yts[g]
            q = 128 * (j % 4)
            for i, a in enumerate(range(NK)):
                rhs = (
                    E[:, EOF[a] + 128 * (j - a) : EOF[a] + 128 * (j - a) + 128]
                    if a <= j
                    else ET[:, ET_OFF[j] + (a - j - 1), :]
                )
                nc.tensor.matmul(
                    out=yt[:, q : q + 128],
                    lhsT=xs_bf[:, a, :],
                    rhs=rhs,
                    start=(i == 0),
                    stop=(i == NK - 1),
                )
            # NOTE: PSUM evacuation must be on DVE/ACT — GPSIMD cannot
            # access PSUM on real hardware (BIR verifier rejects it).
            last_grp = s == BPC - 1 and g == 3
            if last_grp:
                # drain: evacuate per column so the final output chain
                # (evac -> xbar -> combine -> store) starts ASAP
                nc.vector.tensor_copy(
                    out=st[s]["YTsb"][:, 128 * j : 128 * (j + 1)],
                    in_=yt[:, q : q + 128],
                )
                if j % 4 == 3:
                    yts[g] = None
            elif j % 4 == 3:
                nc.vector.tensor_copy(
                    out=st[s]["YTsb"][:, 512 * g : 512 * (g + 1)], in_=yt
                )
                yts[g] = None

        def emit_out(s, g, kcols=None):
            # group g: Y^T slab -> natural layout, combine, store.
            # kcols: emit a single column k (drain mode) instead of the group.
            if not do('all'):
                return
            YTsb, ynat, outsb = st[s]["YTsb"], st[s]["ynat"], st[s]["outsb"]
            f, ax = st[s]["f"], st[s]["ax"]
            ks = kcols if kcols is not None else list(range(4 * g, 4 * (g + 1)))
            c0, c1 = 128 * ks[0], 128 * (ks[-1] + 1)
            nc.sync.dma_start_transpose(
                out=ynat[:, ks[0] : ks[-1] + 1, :],
                in_=YTsb[:, c0:c1],
            )
            for k in ks:
                nc.vector.scalar_tensor_tensor(
                    out=outsb[:, k, :], in0=ynat[:, k, :], scalar=f[:, k : k + 1],
                    in1=ax[:, k, :], op0=MUL, op1=ADD,
                )
            ov = out_ap[s].rearrange("(p k) c -> p k c", p=128)
            dma_legs[g].dma_start(
                out=ov[:, ks[0] : ks[-1] + 1, :], in_=outsb[:, ks[0] : ks[-1] + 1, :]
            )

        # ---- merged pipeline over all samples ----
        emit_front(0)

        if not do('gram'):
            for s in range(1, BPC):
                emit_front(s)
            if do('all'):
                for s in range(BPC):
                    emit_ax(s)
                    st[s]["YTsb"] = sx.tile([64, T], BF16, tag="YTsb", name=f"YTsb_{s}")
                    st[s]["ynat"] = sx.tile([128, NK, C], BF16, tag="ynat", name=f"ynat_{s}")
                    st[s]["outsb"] = sx.tile([128, NK, C], F32, tag="outsb", name=f"outsb_{s}")
                    nc.vector.memset(st[s]["YTsb"], 0.0)
                    for g in range(4):
                        emit_out(s, g)
            return

        # Sample s's pipeline runs at local step j = t - s*SOFF, so
        # consecutive samples overlap by NK-SOFF steps: the next sample's
        # wide early gram rows interleave with this sample's Y tail.
        JMAX = NK + (YSHIFT + 4 if YMODE == "wide" else OSHIFT)
        T_END = (BPC - 1) * SOFF + JMAX
        for t in range(-1, T_END + 1):
            for s in range(BPC):
                j = t - s * SOFF
                if j < -1 or j > JMAX:
                    continue
                if j == -1:
                    alloc_main(s)
                    emit_gram(s, 0)
                    continue
                last = s == BPC - 1
                if j < NK:
                    emit_exp(s, j)
                jy = j - YSHIFT
                if do('y') and 0 <= jy < NK:
                    if YMODE == "wide":
                        emit_ya(s, jy)
                    else:
                        emit_y(s, jy)
                jg = j + 1
                if jg < NK:
                    emit_gram(s, jg)
                if j < NK:
                    emit_xbar(s, j)
                # hooks: per-sample ax + next sample's front-end
                if j == 1:
                    emit_ax(s)
                sn = s + 1
                if sn < BPC:
                    if j == SOFF - 9:
                        emit_sq_mul(sn)
                        emit_cast(sn)
                    elif SOFF - 8 <= j <= SOFF - 5:
                        emit_transp(sn, j - (SOFF - 8))
                    elif j == SOFF - 4:
                        emit_sq_reduce(sn)
                    elif j == SOFF - 3:
                        emit_xsf(sn)
                if do('y'):
                    if YMODE == "wide":
                        # all groups complete at jy == NK-1; stagger the
                        # four output chains over the following steps
                        go = j - (NK - 1 + YSHIFT) - 1
                        if 0 <= go < 4:
                            emit_out(s, go)
                    else:
                        if last:
                            # drain: per-column output chain for the final group
                            jo2 = j - (YSHIFT + 1)
                            if 12 <= jo2 < NK:
                                emit_out(s, 3, [jo2])
                        jo = j - OSHIFT
                        if 0 <= jo < NK and jo % 4 == 3 and not (last and jo >= 12):
                            emit_out(s, jo // 4)

        if do('all') and not do('y'):
            for s in range(BPC):
                nc.vector.memset(st[s]["YTsb"], 0.0)
                for g in range(4):
                    emit_out(s, g)


_NC_CACHE = {}


def _get_nc(reps=1, stages='all'):
    key = (reps, stages)
    if key not in _NC_CACHE:
        _NC_CACHE[key] = build_nc(reps, stages)
    return _NC_CACHE[key]


def _run(x, r_sigma, margin, trace=False, reps=1, stages='all'):
    nc = _get_nc(reps, stages)
    x = np.ascontiguousarray(np.asarray(x, dtype=np.float32))
    r_sigma = np.ascontiguousarray(np.asarray(r_sigma, dtype=np.float32))
    margin = np.ascontiguousarray(np.asarray(margin, dtype=np.float32))
    in_maps = [
        {
            "x": np.ascontiguousarray(x[c * BPC : (c + 1) * BPC]),
            "r_sigma": r_sigma,
            "margin": margin,
        }
        for c in range(N_CORES)
    ]
    res = run_bass_kernel_spmd(nc, in_maps, core_ids=list(range(N_CORES)), trace=trace)
    out = np.concatenate([res.results[c]["out"] for c in range(N_CORES)], axis=0)
    return out, res


def kernel(x, r_sigma, margin):
    out, _ = _run(x, r_sigma, margin, trace=False)
    return out
